# revision 8
# baseline (speedup 1.0000x reference)
"""Cost-volume kernel for Trainium2 (8 NeuronCores, batch-parallel).

Problem: cost[b, o=(dy,dx), h, w] = PReLU(mean_c(c1[b,c,h,w] *
         pad(warped)[b,c,h+dy,w+dx]), alpha), 81 offsets (9x9), zero pad 4.

Strategy per core (one batch element per NeuronCore):
  - Image tiled 16x8 pixels (th x tw), M=128 pixel tile, b-major partition
    order (m = b8*16 + a).
  - TensorE computes a "gram" tile against the 24x16 warped halo:
    PSUM[m, n] = sum_c c1[c, p_m] * wpad[c, halo_n]  (K=96+96 chunks,
    N=384, bf16 inputs, fp32 accumulate).
  - The 81 cost entries of pixel (a, b8) live at n = (a+dy)*16 + (b8+dx),
    a sheared per-partition window that no SBUF AP can express (partition
    steps cannot carry byte remainders), so the device writes the
    partition-uniform superset window [a*16, a*16+144) for each row-group
    a (partitions {a+16*b8}), and the host finishes with a cheap numpy
    diagonal gather + PReLU + 1/192 scale.

Scheduling (v2): wpad is loaded in 4 per-group 40-row chunks so compute
starts after ~2.6MB instead of ~4.6MB; c1 prefetch depth 2; the per-a
output DMAs are split between the sync HWDGE ring and the gpsimd SWDGE
ring so descriptor issue (~0.8us per DMA) doesn't serialize one engine.
"""

import numpy as np

B, C, H, W = 8, 192, 128, 160
R = 4
TH, TW = 16, 8                    # pixel tile
HH, HWW = TH + 2 * R, TW + 2 * R  # halo 24 x 16
NCOL = HH * HWW                   # 384 matmul free dim
BANDS = H // TH                   # 8 row bands
TPB = W // TW                     # 20 tiles per band
WIN = 2 * R * HWW + TW + 2 * R    # 144 per-a superset window
PH, PW = H + 2 * R, W + 2 * R     # padded 136 x 168
K0, K1 = 96, 96                   # contraction chunks
GB = 2                            # bands per staged group
NGRP = BANDS // GB                # 4 staged groups
WROWS = PH // 2 + R               # 72 rows per wpad half (8-row overlap)

_CACHE = {}


def _build():
    if "nc" in _CACHE:
        return _CACHE["nc"]
    import sys
    if "/opt/trn_rl_repo" not in sys.path:
        sys.path.insert(0, "/opt/trn_rl_repo")
    import concourse.mybir as mybir
    import concourse.tile as tile
    from concourse import bacc
    from concourse.bass import AP

    nc = bacc.Bacc(None, target_bir_lowering=False)
    bf16 = mybir.dt.bfloat16
    f32 = mybir.dt.float32

    # c1 pre-tiled on host: [C, band, t, m], m = b8*16 + a
    c1_d = nc.dram_tensor("c1b", [C, H * W], bf16, kind="ExternalInput")
    wp_d = nc.dram_tensor("wpad", [C, PH * PW], bf16, kind="ExternalInput")
    go_d = nc.dram_tensor("gout", [NGRP * TH, TW * GB * TPB * WIN], bf16,
                          kind="ExternalOutput")

    with tile.TileContext(nc) as tc:
        with (
            tc.tile_pool(name="wp", bufs=1) as wp_pool,
            tc.tile_pool(name="c1", bufs=4) as c1_pool,
            tc.tile_pool(name="st", bufs=2) as st_pool,
            tc.tile_pool(name="ps", bufs=4, space="PSUM") as ps_pool,
        ):
            # persistent padded warped: 2 row-halves x 2 channel chunks,
            # each half loaded as three 24-row chunked DMAs so the first
            # band's matmuls only wait on rows [0,24) (range-granular tile
            # dependency tracking), not the whole half.
            wp_sb = {}

            def load_wp(half, eng, rows=(24, 24, 24)):
                row0 = half * (PH - WROWS)  # 0 or 64
                for k, (ks, kn) in enumerate(((0, K0), (K0, K1))):
                    t = wp_pool.tile([kn, WROWS * PW], bf16,
                                     tag=f"wp{half}{k}")
                    wp_sb[(half, k)] = t
                r = 0
                for nrows in rows:
                    for k, (ks, kn) in enumerate(((0, K0), (K0, K1))):
                        t = wp_sb[(half, k)]
                        eng.dma_start(
                            t[:, r * PW:(r + nrows) * PW],
                            wp_d[ks:ks + kn,
                                 (row0 + r) * PW:(row0 + r + nrows) * PW])
                    r += nrows
                assert r == WROWS

            load_wp(0, nc.sync)

            def load_c1(band, eng):
                tiles = []
                for k, (ks, kn) in enumerate(((0, K0), (K0, K1))):
                    t = c1_pool.tile([kn, TPB * 128], bf16, tag=f"c1_{k}")
                    eng.dma_start(
                        t[:], c1_d[ks:ks + kn,
                                   band * TPB * 128:(band + 1) * TPB * 128])
                    tiles.append(t)
                return tiles

            c1_tiles = {0: load_c1(0, nc.scalar)}
            c1_tiles[1] = load_c1(1, nc.scalar)
            load_wp(1, nc.scalar, rows=(48, 24))
            c1_tiles[2] = load_c1(2, nc.scalar)

            # PE warm-up burst during the initial DMA window: back-to-back
            # dummy matmuls keep the HAM clock at 2.4GHz before the real
            # stream starts.
            warm = c1_pool.tile([128, 512], bf16, tag="warm")
            nc.gpsimd.memset(warm[:], 0.0)
            for _ in range(10):
                ps_w = ps_pool.tile([128, 1024], f32, tag="ps")
                nc.tensor.matmul(ps_w[:, 0:512], warm[:, 0:128],
                                 warm[:, 0:512], start=True, stop=True)

            for grp in range(NGRP):
                staged = st_pool.tile([128, GB * TPB * NCOL], bf16,
                                      tag="staged")
                sap0 = staged[:]
                srow = sap0.ap[0][0]

                for bb in range(GB):
                    band = grp * GB + bb
                    r0 = band * TH
                    half = 0 if band < BANDS // 2 else 1
                    prow0 = half * (PH - WROWS)
                    c1_sb = c1_tiles.pop(band)
                    # prefetch three bands ahead of this band's matmuls
                    if band + 3 < BANDS:
                        c1_tiles[band + 3] = load_c1(band + 3, nc.scalar)

                    for tp in range(TPB // 2):
                        ps = ps_pool.tile([128, 1024], f32, tag="ps")
                        for hf in range(2):
                            t_i = 2 * tp + hf
                            c0 = t_i * TW
                            for k, kn in enumerate((K0, K1)):
                                a1 = c1_sb[k][:]
                                lhsT = AP(a1.tensor,
                                          a1.offset + t_i * 128,
                                          [[a1.ap[0][0], kn], [1, 128]])
                                a2 = wp_sb[(half, k)][:]
                                rhs = AP(a2.tensor,
                                         a2.offset + (r0 - prow0) * PW + c0,
                                         [[a2.ap[0][0], kn],
                                          [PW, HH], [1, HWW]])
                                nc.tensor.matmul(
                                    ps[:, hf * 512:hf * 512 + NCOL],
                                    lhsT, rhs,
                                    start=(k == 0), stop=(k == 1))
                        # one copy moves both tiles' grams; DVE/ACT split
                        pap = ps[:]
                        src2 = AP(pap.tensor, pap.offset,
                                  [[pap.ap[0][0], 128], [512, 2],
                                   [1, NCOL]])
                        d0 = (bb * TPB + 2 * tp) * NCOL
                        dst2 = staged[:, d0:d0 + 2 * NCOL]
                        if tp % 5 < 3:
                            nc.vector.tensor_copy(dst2, src2)
                        else:
                            nc.scalar.copy(dst2, src2)

                # per-a out-DMAs, spread across the sync HWDGE and gpsimd
                # SWDGE rings. The last group goes per-band and also uses
                # the scalar ring so its drain starts earlier and the
                # issue cost spreads over three otherwise-idle engines.
                gap = go_d[:]
                rings = (nc.sync, nc.gpsimd)
                if grp == NGRP - 1:
                    rings = (nc.sync, nc.scalar, nc.gpsimd)
                    for bb in range(GB):
                        for a in range(TH):
                            src = AP(sap0.tensor,
                                     sap0.offset + a * srow + a * HWW
                                     + bb * TPB * NCOL,
                                     [[TH * srow, TW], [NCOL, TPB],
                                      [1, WIN]])
                            dst = AP(gap.tensor,
                                     gap.offset + (grp * TH + a)
                                     * (TW * GB * TPB * WIN)
                                     + bb * TPB * WIN,
                                     [[GB * TPB * WIN, TW], [WIN, TPB],
                                      [1, WIN]])
                            rings[a % 3].dma_start(dst, src)
                else:
                    for a in range(TH):
                        src = AP(sap0.tensor,
                                 sap0.offset + a * srow + a * HWW,
                                 [[TH * srow, TW], [NCOL, GB * TPB],
                                  [1, WIN]])
                        dst = AP(gap.tensor,
                                 gap.offset + (grp * TH + a)
                                 * (TW * GB * TPB * WIN),
                                 [[GB * TPB * WIN, TW], [WIN, GB * TPB],
                                  [1, WIN]])
                        rings[a % 2].dma_start(dst, src)

    nc.finalize()
    _CACHE["nc"] = nc
    return nc


def kernel(c1, warped, alpha):
    import sys
    if "/opt/trn_rl_repo" not in sys.path:
        sys.path.insert(0, "/opt/trn_rl_repo")
    import ml_dtypes
    from concourse.bass_utils import run_bass_kernel_spmd

    nc = _build()
    bf = ml_dtypes.bfloat16

    in_maps = []
    for b in range(B):
        wpad = np.zeros((C, PH, PW), np.float32)
        wpad[:, R:R + H, R:R + W] = warped[b]
        # tile c1: [C, band, a, t, b8] -> [C, band, t, b8, a]; m = b8*16 + a
        c1t = np.asarray(c1[b]).reshape(C, BANDS, TH, TPB, TW)
        c1t = np.ascontiguousarray(c1t.transpose(0, 1, 3, 4, 2))
        in_maps.append({
            "c1b": c1t.reshape(C, H * W).astype(bf),
            "wpad": wpad.reshape(C, PH * PW).astype(bf),
        })

    import os
    trace = bool(int(os.environ.get("COSTVOL_TRACE", "0")))
    res = run_bass_kernel_spmd(nc, in_maps, core_ids=list(range(B)),
                               trace=trace)
    if trace:
        _CACHE["last_exec_time_ns"] = res.exec_time_ns

    # host-side: diagonal gather + mean + PReLU
    a_val = float(np.asarray(alpha).reshape(-1)[0])
    dy, dx = np.meshgrid(np.arange(9), np.arange(9), indexing="ij")
    oidx = (dy * HWW + dx).reshape(-1)                      # [81]
    jidx = np.arange(TW)[:, None] + oidx[None, :]           # [b8, 81]

    out = np.empty((B, 81, H, W), np.float32)
    for b in range(B):
        g = np.asarray(res.results[b]["gout"]).astype(np.float32)
        # [grp*16+a, b8, band2, t, j]
        g = g.reshape(NGRP, TH, TW, GB, TPB, WIN)
        got = np.take_along_axis(
            g, jidx[None, None, :, None, None, :], axis=5)
        # -> [81, grp, band2, a, t, b8] -> [81, h, w]
        cost = got.transpose(5, 0, 3, 1, 4, 2).reshape(81, H, W) * (1.0 / C)
        out[b] = np.where(cost >= 0, cost, a_val * cost)
    return out


# revision 10
# speedup vs baseline: 1.1703x; 1.1703x over previous
"""Cost-volume kernel for Trainium2 (8 NeuronCores, batch-parallel).

Problem: cost[b, o=(dy,dx), h, w] = PReLU(mean_c(c1[b,c,h,w] *
         pad(warped)[b,c,h+dy,w+dx]), alpha), 81 offsets (9x9), zero pad 4.

Strategy per core (one batch element per NeuronCore):
  - Image tiled 16x8 pixels (th x tw), M=128 pixel tile, b-major partition
    order (m = b8*16 + a).
  - TensorE computes a "gram" tile against the 24x16 warped halo:
    PSUM[m, n] = sum_c c1[c, p_m] * wpad[c, halo_n]  (K=96+96 chunks,
    N=384, bf16 inputs, fp32 accumulate).
  - The 81 cost entries of pixel (a, b8) live at n = (a+dy)*16 + (b8+dx),
    a sheared per-partition window that no SBUF AP can express (partition
    steps cannot carry byte remainders), so the device writes the
    partition-uniform superset window [a*16, a*16+144) for each row-group
    a (partitions {a+16*b8}), and the host finishes with a cheap numpy
    diagonal gather + PReLU + 1/192 scale.

Scheduling (v4):
  - wpad lives in one SBUF tile per channel chunk (rows 4..132 loaded in
    24-32-row chunks on the sync ring, paced by the band loop; the 4 pad
    rows top/bottom are memset on device). Range-granular tile deps let
    band 0 start after the first 0.8MB.
  - the scalar ring carries only c1 (prefetch depth 3).
  - gout writes go mostly through the gpsimd SWDGE ring: the software DGE
    aggregates the scattered 288B window runs into ~4.3KB packets that
    spread across all 16 DMA engines (HWDGE write packets stay 288B and
    pin to 8 engines). The last group issues per-band on 3 rings so the
    drain tail is short.
"""

import numpy as np

B, C, H, W = 8, 192, 128, 160
R = 4
TH, TW = 16, 8                    # pixel tile
HH, HWW = TH + 2 * R, TW + 2 * R  # halo 24 x 16
NCOL = HH * HWW                   # 384 matmul free dim
BANDS = H // TH                   # 8 row bands
TPB = W // TW                     # 20 tiles per band
WIN = 2 * R * HWW + TW + 2 * R    # 144 per-a superset window
PH, PW = H + 2 * R, W + 2 * R     # padded 136 x 168
K0, K1 = 96, 96                   # contraction chunks
GB = 2                            # bands per staged group
NGRP = BANDS // GB                # 4 staged groups

_CACHE = {}


def _build():
    if "nc" in _CACHE:
        return _CACHE["nc"]
    import sys
    if "/opt/trn_rl_repo" not in sys.path:
        sys.path.insert(0, "/opt/trn_rl_repo")
    import concourse.mybir as mybir
    import concourse.tile as tile
    from concourse import bacc
    from concourse.bass import AP

    nc = bacc.Bacc(None, target_bir_lowering=False)
    bf16 = mybir.dt.bfloat16
    f32 = mybir.dt.float32

    # c1 pre-tiled on host: [C, band, t, m], m = b8*16 + a
    c1_d = nc.dram_tensor("c1b", [C, H * W], bf16, kind="ExternalInput")
    wp_d = nc.dram_tensor("wpad", [C, PH * PW], bf16, kind="ExternalInput")
    go_d = nc.dram_tensor("gout", [NGRP * TH, TW * GB * TPB * WIN], bf16,
                          kind="ExternalOutput")

    with tile.TileContext(nc) as tc:
        with (
            tc.tile_pool(name="wp", bufs=1) as wp_pool,
            tc.tile_pool(name="c1", bufs=4) as c1_pool,
            tc.tile_pool(name="st", bufs=2) as st_pool,
            tc.tile_pool(name="ps", bufs=4, space="PSUM") as ps_pool,
        ):
            # one persistent padded-warped tile per channel chunk; rows
            # [4,132) are real data, rows [0,4) and [132,136) are pad.
            wp_sb = {}
            for k, kn in enumerate((K0, K1)):
                t = wp_pool.tile([kn, PH * PW], bf16, tag=f"wp{k}")
                wp_sb[k] = t
                # zero the 4 pad rows top and bottom (x-pad zeros come in
                # with the loaded rows; host stores them in DRAM)
                nc.gpsimd.memset(wp_sb[k][:, 0:R * PW], 0.0)
                nc.gpsimd.memset(wp_sb[k][:, (PH - R) * PW:PH * PW], 0.0)

            # row chunks: first one small so band 0 unblocks early
            WCHUNKS = ((4, 24), (24, 56), (56, 88), (88, 120), (120, 132))

            def load_wp_chunk(ci):
                ra, rb = WCHUNKS[ci]
                for k, (ks, kn) in enumerate(((0, K0), (K0, K1))):
                    nc.sync.dma_start(
                        wp_sb[k][:, ra * PW:rb * PW],
                        wp_d[ks:ks + kn, ra * PW:rb * PW])

            def load_c1(band, eng):
                tiles = []
                for k, (ks, kn) in enumerate(((0, K0), (K0, K1))):
                    t = c1_pool.tile([kn, TPB * 128], bf16, tag=f"c1_{k}")
                    eng.dma_start(
                        t[:], c1_d[ks:ks + kn,
                                   band * TPB * 128:(band + 1) * TPB * 128])
                    tiles.append(t)
                return tiles

            load_wp_chunk(0)
            load_wp_chunk(1)
            c1_tiles = {0: load_c1(0, nc.scalar)}
            c1_tiles[1] = load_c1(1, nc.scalar)
            load_wp_chunk(2)
            c1_tiles[2] = load_c1(2, nc.scalar)

            # PE warm-up burst during the initial DMA window keeps the HAM
            # clock up before the real stream starts.
            warm = c1_pool.tile([128, 512], bf16, tag="warm")
            nc.gpsimd.memset(warm[:], 0.0)
            for _ in range(10):
                ps_w = ps_pool.tile([128, 1024], f32, tag="ps")
                nc.tensor.matmul(ps_w[:, 0:512], warm[:, 0:128],
                                 warm[:, 0:512], start=True, stop=True)

            for grp in range(NGRP):
                staged = st_pool.tile([128, GB * TPB * NCOL], bf16,
                                      tag="staged")
                sap0 = staged[:]
                srow = sap0.ap[0][0]

                for bb in range(GB):
                    band = grp * GB + bb
                    r0 = band * TH
                    c1_sb = c1_tiles.pop(band)
                    # prefetch three bands ahead; pace wp chunks 3,4
                    if band + 3 < BANDS:
                        c1_tiles[band + 3] = load_c1(band + 3, nc.scalar)
                    if band == 0:
                        load_wp_chunk(3)
                    elif band == 1:
                        load_wp_chunk(4)

                    for tp in range(TPB // 2):
                        ps = ps_pool.tile([128, 1024], f32, tag="ps")
                        for hf in range(2):
                            t_i = 2 * tp + hf
                            c0 = t_i * TW
                            for k, kn in enumerate((K0, K1)):
                                a1 = c1_sb[k][:]
                                lhsT = AP(a1.tensor,
                                          a1.offset + t_i * 128,
                                          [[a1.ap[0][0], kn], [1, 128]])
                                a2 = wp_sb[k][:]
                                rhs = AP(a2.tensor,
                                         a2.offset + r0 * PW + c0,
                                         [[a2.ap[0][0], kn],
                                          [PW, HH], [1, HWW]])
                                nc.tensor.matmul(
                                    ps[:, hf * 512:hf * 512 + NCOL],
                                    lhsT, rhs,
                                    start=(k == 0), stop=(k == 1))
                        # one copy moves both tiles' grams; DVE/ACT split
                        pap = ps[:]
                        src2 = AP(pap.tensor, pap.offset,
                                  [[pap.ap[0][0], 128], [512, 2],
                                   [1, NCOL]])
                        d0 = (bb * TPB + 2 * tp) * NCOL
                        dst2 = staged[:, d0:d0 + 2 * NCOL]
                        if tp % 2 == 0:
                            nc.vector.tensor_copy(dst2, src2)
                        else:
                            nc.scalar.copy(dst2, src2)

                # per-a out-DMAs. Groups 0-2: mostly gpsimd SWDGE (big
                # aggregated packets over all 16 DMA engines), 4/16 on
                # sync. Last group: per-band over 3 rings for a short tail.
                gap = go_d[:]
                if grp == NGRP - 1:
                    rings = (nc.gpsimd, nc.sync, nc.scalar)
                    for bb in range(GB):
                        for a in range(TH):
                            src = AP(sap0.tensor,
                                     sap0.offset + a * srow + a * HWW
                                     + bb * TPB * NCOL,
                                     [[TH * srow, TW], [NCOL, TPB],
                                      [1, WIN]])
                            dst = AP(gap.tensor,
                                     gap.offset + (grp * TH + a)
                                     * (TW * GB * TPB * WIN)
                                     + bb * TPB * WIN,
                                     [[GB * TPB * WIN, TW], [WIN, TPB],
                                      [1, WIN]])
                            rings[a % 3].dma_start(dst, src)
                else:
                    for a in range(TH):
                        src = AP(sap0.tensor,
                                 sap0.offset + a * srow + a * HWW,
                                 [[TH * srow, TW], [NCOL, GB * TPB],
                                  [1, WIN]])
                        dst = AP(gap.tensor,
                                 gap.offset + (grp * TH + a)
                                 * (TW * GB * TPB * WIN),
                                 [[GB * TPB * WIN, TW], [WIN, GB * TPB],
                                  [1, WIN]])
                        eng = nc.sync if a % 4 == 0 else nc.gpsimd
                        eng.dma_start(dst, src)

    nc.finalize()
    _CACHE["nc"] = nc
    return nc


def kernel(c1, warped, alpha):
    import sys
    if "/opt/trn_rl_repo" not in sys.path:
        sys.path.insert(0, "/opt/trn_rl_repo")
    import ml_dtypes
    from concourse.bass_utils import run_bass_kernel_spmd

    nc = _build()
    bf = ml_dtypes.bfloat16

    in_maps = []
    for b in range(B):
        wpad = np.zeros((C, PH, PW), np.float32)
        wpad[:, R:R + H, R:R + W] = warped[b]
        # tile c1: [C, band, a, t, b8] -> [C, band, t, b8, a]; m = b8*16 + a
        c1t = np.asarray(c1[b]).reshape(C, BANDS, TH, TPB, TW)
        c1t = np.ascontiguousarray(c1t.transpose(0, 1, 3, 4, 2))
        in_maps.append({
            "c1b": c1t.reshape(C, H * W).astype(bf),
            "wpad": wpad.reshape(C, PH * PW).astype(bf),
        })

    import os
    trace = bool(int(os.environ.get("COSTVOL_TRACE", "0")))
    res = run_bass_kernel_spmd(nc, in_maps, core_ids=list(range(B)),
                               trace=trace)
    if trace:
        _CACHE["last_exec_time_ns"] = res.exec_time_ns

    # host-side: diagonal gather + mean + PReLU
    a_val = float(np.asarray(alpha).reshape(-1)[0])
    dy, dx = np.meshgrid(np.arange(9), np.arange(9), indexing="ij")
    oidx = (dy * HWW + dx).reshape(-1)                      # [81]
    jidx = np.arange(TW)[:, None] + oidx[None, :]           # [b8, 81]

    out = np.empty((B, 81, H, W), np.float32)
    for b in range(B):
        g = np.asarray(res.results[b]["gout"]).astype(np.float32)
        # [grp*16+a, b8, band2, t, j]
        g = g.reshape(NGRP, TH, TW, GB, TPB, WIN)
        got = np.take_along_axis(
            g, jidx[None, None, :, None, None, :], axis=5)
        # -> [81, grp, band2, a, t, b8] -> [81, h, w]
        cost = got.transpose(5, 0, 3, 1, 4, 2).reshape(81, H, W) * (1.0 / C)
        out[b] = np.where(cost >= 0, cost, a_val * cost)
    return out
